# revision 2
# baseline (speedup 1.0000x reference)
"""2-layer GCN (GCNConv x2 + mean-pool + linear) on 8 Trainium2 NeuronCores.

Strategy (one SPMD launch, all 8 cores):
  - Nodes are renumbered and sharded: 8 cores x 6400 slots (6250 real +
    150 zero pads); per-core slot order sorts by in-degree so that edge
    chunks (one edge per slot per chunk) pad minimally.
  - Per-edge normalization dinv[src]*dinv[dst] is split: dinv[src] is folded
    into the gathered feature table, dinv[dst] is applied per-partition /
    per-column after accumulation. No per-edge scaling on device.
  - P1: every core computes the FULL table1 = (dinv*x) @ W1 (bf16,
    node-major in SBUF).
  - P2: layer-1 propagation: SBUF-source dma_gather of 256B rows (bf16)
    into feat-major message tiles; PE identity-matmuls accumulate chunks
    into [128, 512] PSUM supergroup tiles; post = dinv*relu(dinv*psum+b1).
  - P3: y2 = h1' @ W2 per 128-token tile -> node-major table2 shard.
  - P4: AllGather of the 1.6MB shards -> DRAM table2 [51200, 128].
  - P5: layer-2 propagation gathers straight from the AllGather output
    (HBM-source dma_gather, node-major messages), same PE accumulation;
    post = relu(dinv*psum + b2) -> node-major h2.
  - Mean-pool + final linear run on host (0.03% of the FLOPs).
"""

import numpy as np
import ml_dtypes

N = 50000
E = 800000
D = 128
G = 256
NCORES = 8
SLOTS = 6400
REAL = 6250
GROUP = 128
SG = 4
NGROUPS = SLOTS // GROUP          # 50
NSG = (NGROUPS + SG - 1) // SG    # 13
HALF_TOK = 25600
CALL_CHUNKS = 64
NT = NCORES * SLOTS               # 51200
XSLAB = 2048
PAD_A = SLOTS - 1
PAD_B = 4 * SLOTS + SLOTS - 1 - HALF_TOK
BF16 = ml_dtypes.bfloat16


# --------------------------------------------------------------------------
# host-side scheduling
# --------------------------------------------------------------------------

def _build_schedule(edge_index):
    src0 = np.asarray(edge_index[0], np.int64)
    dst0 = np.asarray(edge_index[1], np.int64)
    loop = np.arange(N, dtype=np.int64)
    src = np.concatenate([src0, loop])
    dst = np.concatenate([dst0, loop])
    deg = np.bincount(dst, minlength=N).astype(np.float64)
    dinv = (1.0 / np.sqrt(deg)).astype(np.float32)

    order = np.argsort(-deg, kind="stable")
    core_of = np.empty(N, np.int32)
    core_of[order] = np.arange(N, dtype=np.int64) % NCORES
    halfB = core_of >= 4
    degA = np.bincount(dst[~halfB[src]], minlength=N)
    degB = np.bincount(dst[halfB[src]], minlength=N)

    slot_of = np.empty(N, np.int64)
    for c in range(NCORES):
        nodes = np.where(core_of == c)[0]
        k = np.lexsort((-(degA[nodes] - degB[nodes]),
                        -np.maximum(degA[nodes], degB[nodes])))
        nodes = nodes[k]
        slot_of[nodes] = np.arange(len(nodes), dtype=np.int64)

    new_id = core_of.astype(np.int64) * SLOTS + slot_of
    tok_node = np.full(NT, -1, np.int64)
    tok_node[new_id] = np.arange(N, dtype=np.int64)

    e_core = core_of[dst]
    e_slot = slot_of[dst]
    e_half = halfB[src].astype(np.int64)
    e_group = e_slot // GROUP
    e_tok = new_id[src]

    e_key = (e_core.astype(np.int64) * SLOTS + e_slot) * 2 + e_half
    eorder = np.argsort(e_key, kind="stable")
    sk = e_key[eorder]
    starts = np.r_[0, np.nonzero(np.diff(sk))[0] + 1]
    lens = np.diff(np.r_[starts, len(sk)])
    seq_sorted = np.arange(len(sk)) - np.repeat(starts, lens)
    e_seq = np.empty(len(sk), np.int64)
    e_seq[eorder] = seq_sorted

    C = np.zeros((NCORES, NGROUPS, 2), np.int64)
    np.maximum.at(C, (e_core, e_group, e_half), e_seq + 1)
    Cg = C.max(axis=0)
    empty = Cg.sum(axis=1) == 0
    Cg[empty, 0] = 1          # every group's PSUM columns must get written

    chunk_off = np.zeros((NGROUPS, 2), np.int64)
    pos = [0, 0]
    sg_chunk_ranges = [[], []]
    for sgi in range(NSG):
        gs = range(sgi * SG, min((sgi + 1) * SG, NGROUPS))
        for h in (0, 1):
            st = pos[h]
            for g in gs:
                chunk_off[g, h] = pos[h]
                pos[h] += Cg[g, h]
            sg_chunk_ranges[h].append((st, pos[h], sgi))
    LCH = (pos[0], pos[1])
    LA, LB = pos[0] * GROUP, pos[1] * GROUP

    idxA = np.full((NCORES, LA), PAD_A, np.int32)
    idxB = np.full((NCORES, LB), PAD_B, np.int32)
    e_chunk = chunk_off[e_group, e_half] + e_seq
    e_pos = e_chunk * GROUP + (e_slot % GROUP)
    mA = e_half == 0
    idxA[e_core[mA], e_pos[mA]] = e_tok[mA]
    idxB[e_core[~mA], e_pos[~mA]] = e_tok[~mA] - HALF_TOK

    def to_r2(tok):
        c = tok // SLOTS
        rem = tok % SLOTS
        st = rem // 128
        p = rem % 128
        return c * SLOTS + p * (SLOTS // 128) + st

    idx2A = to_r2(idxA.astype(np.int64)).astype(np.int32)
    idx2B = (to_r2(idxB.astype(np.int64) + HALF_TOK) - HALF_TOK).astype(np.int32)

    chunk_g = [np.empty(LCH[0], np.int64), np.empty(LCH[1], np.int64)]
    for h in (0, 1):
        for g in range(NGROUPS):
            o = chunk_off[g, h]
            chunk_g[h][o:o + Cg[g, h]] = g
    calls = []
    for sgi in range(NSG):
        for h in (0, 1):
            st, en, _ = sg_chunk_ranges[h][sgi]
            c = st
            while c < en:
                n = min(CALL_CHUNKS, en - c)
                calls.append(dict(half=h, chunk0=c, nchunks=n, sg=sgi))
                c += n

    return dict(dinv=dinv, tok_node=tok_node, new_id=new_id,
                idxA=idxA, idxB=idxB, idx2A=idx2A, idx2B=idx2B,
                chunk_g=chunk_g, calls=calls, LA=LA, LB=LB)


def _wrap_idx16(idx):
    base = idx.reshape(-1, 16).T.astype(np.int16)
    return np.tile(base, (8, 1))


# --------------------------------------------------------------------------
# device kernel
# --------------------------------------------------------------------------

def _build_nc(s):
    import concourse.mybir as mybir
    import concourse.tile as tile
    from concourse import bacc

    calls = s["calls"]
    chunk_g = s["chunk_g"]
    LA, LB = s["LA"], s["LB"]
    gsz_sg = [min((sgi + 1) * SG, NGROUPS) - sgi * SG for sgi in range(NSG)]
    sg_first = {}
    sg_last = {}
    for call in calls:
        sg_first.setdefault(call["sg"], call)
        sg_last[call["sg"]] = call

    nc = bacc.Bacc(None, target_bir_lowering=False, num_devices=NCORES)
    fp32 = mybir.dt.float32
    bf16 = mybir.dt.bfloat16
    i16 = mybir.dt.int16

    xT_d = nc.dram_tensor("xT", [128, NT], bf16, kind="ExternalInput")
    w1_d = nc.dram_tensor("w1", [128, 128], bf16, kind="ExternalInput")
    w2_d = nc.dram_tensor("w2", [128, 128], bf16, kind="ExternalInput")
    id_d = nc.dram_tensor("ident", [128, 128], bf16, kind="ExternalInput")
    b1_d = nc.dram_tensor("b1", [128, 1], fp32, kind="ExternalInput")
    b2bc_d = nc.dram_tensor("b2bc", [128, 128], fp32, kind="ExternalInput")
    dinvb_d = nc.dram_tensor("dinvb", [128, SLOTS], bf16, kind="ExternalInput")
    dinvg_d = nc.dram_tensor("dinvg", [128, NGROUPS], fp32, kind="ExternalInput")
    idxA_d = nc.dram_tensor("idxA", [128, LA // 16], i16, kind="ExternalInput")
    idxB_d = nc.dram_tensor("idxB", [128, LB // 16], i16, kind="ExternalInput")
    idx2A_d = nc.dram_tensor("idx2A", [128, LA // 16], i16, kind="ExternalInput")
    idx2B_d = nc.dram_tensor("idx2B", [128, LB // 16], i16, kind="ExternalInput")
    h2_d = nc.dram_tensor("h2", [128, SLOTS], bf16, kind="ExternalOutput")

    with tile.TileContext(nc) as tc:
        with (
            tc.tile_pool(name="const", bufs=1) as constp,
            tc.tile_pool(name="xslab", bufs=2) as xslabp,
            tc.tile_pool(name="table", bufs=1) as tablep,
            tc.tile_pool(name="idx", bufs=1) as idxp,
            tc.tile_pool(name="msg", bufs=2) as msgp,
            tc.tile_pool(name="ppsum", bufs=3, space="PSUM") as ppsump,
            tc.tile_pool(name="gpsum", bufs=2, space="PSUM") as gpsump,
            tc.tile_pool(name="post", bufs=2) as postp,
            tc.tile_pool(name="hbuf", bufs=1) as hbufp,
            tc.tile_pool(name="dram", bufs=1, space="DRAM") as dramp,
        ):
            w1_sb = constp.tile([128, 128], bf16, tag="w1")
            w2_sb = constp.tile([128, 128], bf16, tag="w2")
            id_sb = constp.tile([128, 128], bf16, tag="ident")
            b1_sb = constp.tile([128, 1], fp32, tag="b1")
            b2bc_sb = constp.tile([128, 128], fp32, tag="b2bc")
            dinvb_sb = constp.tile([128, SLOTS], bf16, tag="dinvb")
            dinvg_sb = constp.tile([128, NGROUPS], fp32, tag="dinvg")
            nc.sync.dma_start(w1_sb[:], w1_d[:])
            nc.sync.dma_start(w2_sb[:], w2_d[:])
            nc.sync.dma_start(id_sb[:], id_d[:])
            nc.sync.dma_start(b1_sb[:], b1_d[:])
            nc.sync.dma_start(b2bc_sb[:], b2bc_d[:])
            nc.sync.dma_start(dinvb_sb[:], dinvb_d[:])
            nc.sync.dma_start(dinvg_sb[:], dinvg_d[:])
            idxA_sb = idxp.tile([128, LA // 16], i16, tag="idxA")
            idxB_sb = idxp.tile([128, LB // 16], i16, tag="idxB")
            nc.sync.dma_start(idxA_sb[:], idxA_d[:])
            nc.sync.dma_start(idxB_sb[:], idxB_d[:])

            table_sb = tablep.tile([128, NT], bf16, tag="table")

            # ---- P1: full-local table1 GEMM ----
            for sl in range(NT // XSLAB):
                xs = xslabp.tile([128, XSLAB], bf16, tag="xs")
                nc.sync.dma_start(xs[:], xT_d[:, sl * XSLAB:(sl + 1) * XSLAB])
                for q in range(XSLAB // 512):
                    ps = gpsump.tile([128, 512], fp32, tag="gps", name="gps")
                    for i in range(4):
                        o = q * 512 + i * 128
                        nc.tensor.matmul(
                            ps[:, i * 128:(i + 1) * 128],
                            lhsT=xs[:, o:o + 128], rhs=w1_sb[:],
                            start=(i == 0), stop=(i == 3))
                    nc.scalar.activation(
                        table_sb[:, sl * XSLAB + q * 512:
                                 sl * XSLAB + (q + 1) * 512],
                        ps[:], mybir.ActivationFunctionType.Copy)

            # ---- P2: layer-1 prop (feat-major, SBUF-source gather) ----
            h1p_sb = hbufp.tile([128, SLOTS], bf16, tag="hbig")
            halves = (table_sb[:, :HALF_TOK], table_sb[:, HALF_TOK:])
            idxs = (idxA_sb, idxB_sb)
            sg_psum = {}
            for call in calls:
                h, c0, nch, sgi = (call["half"], call["chunk0"],
                                   call["nchunks"], call["sg"])
                if sgi not in sg_psum:
                    sg_psum[sgi] = ppsump.tile([128, 512], fp32,
                                               tag="pps", name="pps")
                ps = sg_psum[sgi]
                n = nch * GROUP
                msg = msgp.tile([128, 1, CALL_CHUNKS * GROUP], bf16, tag="msg")
                nc.gpsimd.dma_gather(
                    msg[:, :, :n], halves[h],
                    idxs[h][:, c0 * (GROUP // 16): (c0 + nch) * (GROUP // 16)],
                    n, n, 128,
                    transpose=True, sbuf_tokens_per_rank=128,
                    sbuf_free_dim_per_rank=256, single_packet=False)
                for k in range(nch):
                    gc = int(chunk_g[h][c0 + k]) % SG
                    nc.tensor.matmul(
                        ps[:, gc * 128:(gc + 1) * 128],
                        lhsT=id_sb[:],
                        rhs=msg[:, 0, k * GROUP:(k + 1) * GROUP],
                        start=(sg_first[sgi] is call and k == 0),
                        stop=(sg_last[sgi] is call and k == nch - 1))
                if sg_last[sgi] is call:
                    ncols = gsz_sg[sgi] * 128
                    off = sgi * SG * 128
                    tmp = postp.tile([128, 512], fp32, tag="tmp", name="tmp")
                    nc.vector.tensor_tensor(
                        out=tmp[:, :ncols], in0=ps[:, :ncols],
                        in1=dinvb_sb[:, off:off + ncols],
                        op=mybir.AluOpType.mult)
                    u = postp.tile([128, 512], bf16, tag="u", name="u")
                    nc.scalar.activation(
                        u[:, :ncols], tmp[:, :ncols],
                        mybir.ActivationFunctionType.Relu, bias=b1_sb[:, 0:1])
                    nc.vector.tensor_tensor(
                        out=h1p_sb[:, off:off + ncols], in0=u[:, :ncols],
                        in1=dinvb_sb[:, off:off + ncols],
                        op=mybir.AluOpType.mult)

            # ---- P3: GEMM2 -> node-major table2 shard ----
            shard_sb = hbufp.tile([128, SLOTS], bf16, tag="shard")
            for q in range(NSG):
                ncols = min(512, SLOTS - q * 512)
                ps = gpsump.tile([128, 512], fp32, tag="gps", name="gps")
                ntl = ncols // 128
                for i in range(ntl):
                    o = q * 512 + i * 128
                    nc.tensor.matmul(
                        ps[:, i * 128:(i + 1) * 128],
                        lhsT=h1p_sb[:, o:o + 128], rhs=w2_sb[:],
                        start=(i == 0), stop=(i == ntl - 1))
                nc.scalar.activation(
                    shard_sb[:, q * 512: q * 512 + ncols], ps[:, :ncols],
                    mybir.ActivationFunctionType.Copy)

            # ---- P4: AllGather shards -> DRAM table2 [51200, 128] ----
            ag_in = dramp.tile([128, SLOTS], bf16, tag="ag_in")
            ag_out = dramp.tile([NT, 128], bf16, tag="ag_out",
                                addr_space="Shared")
            nc.sync.dma_start(ag_in[:], shard_sb[:])
            nc.gpsimd.collective_compute(
                "AllGather", mybir.AluOpType.bypass,
                replica_groups=[list(range(NCORES))],
                ins=[ag_in.opt()], outs=[ag_out.opt()])

            # ---- P5: layer-2 prop (node-major, HBM-source gather) ----
            idx2A_sb = idxp.tile([128, LA // 16], i16, tag="idxA", name="idx2A")
            idx2B_sb = idxp.tile([128, LB // 16], i16, tag="idxB", name="idx2B")
            nc.sync.dma_start(idx2A_sb[:], idx2A_d[:])
            nc.sync.dma_start(idx2B_sb[:], idx2B_d[:])
            h2_sb = hbufp.tile([128, SLOTS], bf16, tag="hbig")
            ihalves = (ag_out[:HALF_TOK, :], ag_out[HALF_TOK:, :])
            idxs2 = (idx2A_sb, idx2B_sb)
            sg_psum = {}
            for call in calls:
                h, c0, nch, sgi = (call["half"], call["chunk0"],
                                   call["nchunks"], call["sg"])
                if sgi not in sg_psum:
                    sg_psum[sgi] = ppsump.tile([128, 512], fp32,
                                               tag="pps", name="pps")
                ps = sg_psum[sgi]
                n = nch * GROUP
                msg = msgp.tile([128, CALL_CHUNKS, GROUP], bf16, tag="msg")
                nc.gpsimd.dma_gather(
                    msg[:, :nch, :], ihalves[h],
                    idxs2[h][:, c0 * (GROUP // 16): (c0 + nch) * (GROUP // 16)],
                    n, n, 128, single_packet=False)
                for k in range(nch):
                    gc = int(chunk_g[h][c0 + k]) % SG
                    nc.tensor.matmul(
                        ps[:, gc * 128:(gc + 1) * 128],
                        lhsT=id_sb[:], rhs=msg[:, k, :],
                        start=(sg_first[sgi] is call and k == 0),
                        stop=(sg_last[sgi] is call and k == nch - 1))
                if sg_last[sgi] is call:
                    gs = gsz_sg[sgi]
                    ncols = gs * 128
                    off = sgi * SG * 128
                    tmp = postp.tile([128, 512], fp32, tag="tmp", name="tmp")
                    nc.vector.tensor_tensor(
                        out=tmp[:, :ncols], in0=ps[:, :ncols],
                        in1=dinvg_sb[:, sgi * SG: sgi * SG + gs]
                            .unsqueeze(2).to_broadcast([128, gs, 128]),
                        op=mybir.AluOpType.mult)
                    nc.vector.tensor_tensor(
                        out=tmp[:, :ncols], in0=tmp[:, :ncols],
                        in1=b2bc_sb[:].unsqueeze(1).to_broadcast([128, gs, 128]),
                        op=mybir.AluOpType.add)
                    nc.scalar.activation(
                        h2_sb[:, off:off + ncols], tmp[:, :ncols],
                        mybir.ActivationFunctionType.Relu)
            nc.sync.dma_start(h2_d[:], h2_sb[:])

    nc.compile()
    return nc


# --------------------------------------------------------------------------
# top-level entry
# --------------------------------------------------------------------------

_cached = None
last_results = None


def kernel(x, edge_index, batch, W1, b1, W2, b2, Wl, bl, trace=False):
    global _cached, last_results
    x = np.asarray(x, np.float32)
    edge_index = np.asarray(edge_index)
    batch = np.asarray(batch, np.int64)
    W1 = np.asarray(W1, np.float32)
    b1 = np.asarray(b1, np.float32)
    W2 = np.asarray(W2, np.float32)
    b2 = np.asarray(b2, np.float32)
    Wl = np.asarray(Wl, np.float32)
    bl = np.asarray(bl, np.float32)
    assert x.shape == (N, D) and edge_index.shape == (2, E)

    ekey = hash(edge_index.tobytes())
    if _cached is None or _cached[0] != ekey:
        s = _build_schedule(edge_index)
        nc = _build_nc(s)
        _cached = (ekey, s, nc)
    _, s, nc = _cached

    dinv, tok_node = s["dinv"], s["tok_node"]
    valid = tok_node >= 0

    xT = np.zeros((NT, D), np.float32)
    xT[valid] = x[tok_node[valid]] * dinv[tok_node[valid]][:, None]
    xT_bf = np.ascontiguousarray(xT.T).astype(BF16)

    dinv_tok = np.zeros(NT, np.float32)
    dinv_tok[valid] = dinv[tok_node[valid]]

    common = {
        "xT": xT_bf,
        "w1": W1.astype(BF16),
        "w2": W2.astype(BF16),
        "ident": np.eye(128, dtype=BF16),
        "b1": b1.reshape(128, 1).astype(np.float32),
        "b2bc": np.ascontiguousarray(
            np.broadcast_to(b2, (128, 128))).astype(np.float32),
    }
    in_maps = []
    for c in range(NCORES):
        dv = dinv_tok[c * SLOTS:(c + 1) * SLOTS]
        m = dict(common)
        m["dinvb"] = np.ascontiguousarray(
            np.broadcast_to(dv.astype(BF16), (128, SLOTS)))
        m["dinvg"] = np.ascontiguousarray(
            dv.reshape(NGROUPS, 128).T).astype(np.float32)
        m["idxA"] = _wrap_idx16(s["idxA"][c])
        m["idxB"] = _wrap_idx16(s["idxB"][c])
        m["idx2A"] = _wrap_idx16(s["idx2A"][c])
        m["idx2B"] = _wrap_idx16(s["idx2B"][c])
        in_maps.append(m)

    from concourse.bass_utils import run_bass_kernel_spmd
    res = run_bass_kernel_spmd(nc, in_maps, list(range(NCORES)), trace=trace)
    last_results = res

    h2_full = np.concatenate(
        [np.asarray(r["h2"], dtype=BF16).astype(np.float32)
         .reshape(128, NGROUPS, 128).transpose(1, 0, 2).reshape(SLOTS, D)
         for r in res.results], axis=0)
    h2 = np.zeros((N, D), np.float32)
    h2[tok_node[valid]] = h2_full[valid]
    sums = np.zeros((G, D), np.float32)
    np.add.at(sums, batch, h2)
    cnts = np.bincount(batch, minlength=G).astype(np.float32)
    pooled = sums / np.maximum(cnts, 1.0)[:, None]
    return (pooled @ Wl + bl).astype(np.float32)


def modeled_exec_time_ns():
    """Cost-model (TimelineSim) estimate for the compiled kernel, in ns."""
    if _cached is None:
        return None
    from concourse.timeline_sim import TimelineSim
    return int(TimelineSim(_cached[2], trace=False).simulate())
